# revision 11
# baseline (speedup 1.0000x reference)
"""CrossAttentionBlock TRN2 kernel.

Full inputs -> shard batch dim over 8 NeuronCores (data parallel, 4 batches
each) -> Bass/Tile kernel per core -> gather outputs.

Shapes (hardcoded): x [32,512,32,32] f32, t [32,77,768] f32,
Wq [512,512], Wkv [1024,768], Wp [512,512]; out [32,512,32,32].

Per-core plan (B_local=4, C=512, HW=1024, L=77, D=768, heads=8, hd=64):
  GroupNorm(32 groups): channel-on-partition layout [128,1024] x4 tiles;
    bn_stats per channel, group aggregation + per-channel expansion via tiny
    PE matmuls with constant selection matrices (host-provided).
  q = WqT.T @ xn        (f32r matmuls, N=512)
  LayerNorm(t): [77,768] row layout, then PE-transpose to [768,77].
  kv = t_lnT.T @ WkvT   -> [77, 1024] (l on partitions)
  per head h: k = kv[:, 128h:128h+64] transposed to [64,77] (PE),
    v stays [77(s), 64(c)] = natural lhsT for the AV matmul.
    attnT[s,t] = k.T @ q_head     ([77,512] x2, f32r)
    exp = Exp(0.125 * attnT)      (ACT, psum->sbuf; max-sub skipped, fp32 safe)
    hU[c,t]   = v.T @ exp         (f32r)  -- unnormalized
    den[c,t]  = ones.T @ exp      (f32r)  -- softmax denom broadcast to 64 rows
    h = hU * recip(den)           (DVE)
  out = WpT.T @ h + bp + x        (f32r matmuls; bias+residual on DVE/GPSIMD)
"""

import os
import sys

import numpy as np

for _p in ("/opt/trn_rl_repo", "/root/.axon_site/_ro/trn_rl_repo"):
    if _p not in sys.path and os.path.isdir(_p):
        sys.path.append(_p)

import concourse.bass as bass
import concourse.tile as tile
from concourse import bacc, mybir
from concourse.bass_utils import run_bass_kernel_spmd

F32 = mybir.dt.float32
F32R = mybir.dt.float32r
EPS = 1e-5

N_CORES = 8
B, C, H, W = 32, 512, 32, 32
HW = H * W
L, D = 77, 768
NH, HD = 8, 64
NG, GS = 32, 16  # groups, channels per group
BL = B // N_CORES  # local batches per core

LAST_RESULTS = None
_CACHE = {}
USE_DIVIDE = False


def R(ap):
    return ap.bitcast(F32R)


def _build_program():
    nc = bacc.Bacc("TRN2", target_bir_lowering=False, debug=False)

    x_l = nc.declare_dram_parameter("x_l", [BL, C, HW], F32, isOutput=False)
    t_l = nc.declare_dram_parameter("t_l", [BL, L, D], F32, isOutput=False)
    wqt = nc.declare_dram_parameter("wqt", [C, C], F32R, isOutput=False)
    wkvt = nc.declare_dram_parameter("wkvt", [D, 2 * C], F32R, isOutput=False)
    wpt = nc.declare_dram_parameter("wpt", [C, C], F32R, isOutput=False)
    gnw4 = nc.declare_dram_parameter("gnw4", [128, 4], F32, isOutput=False)
    gnb4 = nc.declare_dram_parameter("gnb4", [128, 4], F32, isOutput=False)
    bp4 = nc.declare_dram_parameter("bp4", [128, 4], F32, isOutput=False)
    lnw1 = nc.declare_dram_parameter("lnw1", [1, D], F32, isOutput=False)
    lnb1 = nc.declare_dram_parameter("lnb1", [1, D], F32, isOutput=False)
    gsel = nc.declare_dram_parameter("gsel", [128, 8], F32, isOutput=False)
    gselt = nc.declare_dram_parameter("gselt", [8, 128], F32, isOutput=False)
    ident = nc.declare_dram_parameter("ident", [128, 128], F32, isOutput=False)
    ones64 = nc.declare_dram_parameter("ones64", [128, 64], F32R, isOutput=False)
    out_l = nc.declare_dram_parameter("out_l", [BL, C, HW], F32, isOutput=True)

    TT = mybir.AluOpType

    with tile.TileContext(nc) as tc:
        with (
            tc.tile_pool(name="consts", bufs=1) as consts,
            tc.tile_pool(name="xp", bufs=5) as xp,
            tc.tile_pool(name="xnp", bufs=4) as xnp,
            tc.tile_pool(name="qp", bufs=5) as qpool,
            tc.tile_pool(name="hp", bufs=4) as hpool,
            tc.tile_pool(name="op", bufs=3) as opool,
            tc.tile_pool(name="tp", bufs=2) as tpool,
            tc.tile_pool(name="kvp", bufs=2) as kvpool,
            tc.tile_pool(name="ktp", bufs=6) as ktpool,
            tc.tile_pool(name="ttp", bufs=2) as ttpool,
            tc.tile_pool(name="exp", bufs=2) as expool,
            tc.tile_pool(name="rcp", bufs=2) as rcpool,
            tc.tile_pool(name="sp", bufs=4) as spool,
            tc.tile_pool(name="abp", bufs=6) as abpool,
            tc.tile_pool(name="psmm", bufs=6, space="PSUM") as psmm,
            tc.tile_pool(name="pstr", bufs=2, space="PSUM") as pstr,
        ):
            # ---- constants ----
            wqt_sb = []
            wpt_sb = []
            for ki in range(4):
                tq = consts.tile([128, C], F32R, tag=f"wqt{ki}")
                nc.sync.dma_start(out=tq, in_=wqt[128 * ki : 128 * (ki + 1), :])
                wqt_sb.append(tq)
                tp_ = consts.tile([128, C], F32R, tag=f"wpt{ki}")
                nc.sync.dma_start(out=tp_, in_=wpt[128 * ki : 128 * (ki + 1), :])
                wpt_sb.append(tp_)
            wkvt_sb = []
            for di in range(6):
                tk = consts.tile([128, 2 * C], F32R, tag=f"wkvt{di}")
                nc.sync.dma_start(out=tk, in_=wkvt[128 * di : 128 * (di + 1), :])
                wkvt_sb.append(tk)
            gnw_sb = consts.tile([128, 4], F32, tag="gnw")
            nc.sync.dma_start(out=gnw_sb, in_=gnw4[:, :])
            gnb_sb = consts.tile([128, 4], F32, tag="gnb")
            nc.sync.dma_start(out=gnb_sb, in_=gnb4[:, :])
            bp_sb = consts.tile([128, 4], F32, tag="bp")
            nc.sync.dma_start(out=bp_sb, in_=bp4[:, :])
            lnw_sb = consts.tile([128, D], F32, tag="lnw")
            nc.sync.dma_start(out=lnw_sb, in_=lnw1[:, :].to_broadcast([128, D]))
            lnb_sb = consts.tile([128, D], F32, tag="lnb")
            nc.sync.dma_start(out=lnb_sb, in_=lnb1[:, :].to_broadcast([128, D]))
            gsel_sb = consts.tile([128, 8], F32, tag="gsel")
            nc.sync.dma_start(out=gsel_sb, in_=gsel[:, :])
            gselt_sb = consts.tile([8, 128], F32, tag="gselt")
            nc.sync.dma_start(out=gselt_sb, in_=gselt[:, :])
            ident_sb = consts.tile([128, 128], F32, tag="ident")
            nc.sync.dma_start(out=ident_sb, in_=ident[:, :])
            ones_sb = consts.tile([128, 64], F32R, tag="ones64")
            nc.sync.dma_start(out=ones_sb, in_=ones64[:, :])
            eps_sb = consts.tile([128, 1], F32, tag="eps")
            nc.vector.memset(eps_sb, EPS)

            for b in range(BL):
                # ======== Phase X: load x, GroupNorm stats ========
                xb = []
                for i in range(4):
                    xt = xp.tile([128, HW], F32, tag="x")
                    nc.sync.dma_start(
                        out=xt, in_=x_l[b, 128 * i : 128 * (i + 1), :]
                    )
                    xb.append(xt)

                mv = spool.tile([128, 4, 2], F32, tag="mv")
                for i in range(4):
                    st = spool.tile([128, 2, 6], F32, tag="bnst")
                    for j in range(2):
                        nc.vector.bn_stats(
                            out=st[:, j, :], in_=xb[i][:, 512 * j : 512 * (j + 1)]
                        )
                    nc.vector.bn_aggr(out=mv[:, i, :], in_=st)

                # me2[:, i, 0]=mean_c ; me2[:, i, 1]=var_c+mean_c^2 (=E[x^2]_c)
                me2 = spool.tile([128, 4, 2], F32, tag="me2")
                nc.vector.tensor_copy(out=me2[:, :, 0], in_=mv[:, :, 0])
                nc.vector.tensor_tensor(
                    out=me2[:, :, 1], in0=mv[:, :, 0], in1=mv[:, :, 0], op=TT.mult
                )
                nc.vector.tensor_tensor(
                    out=me2[:, :, 1], in0=me2[:, :, 1], in1=mv[:, :, 1], op=TT.add
                )
                gpsum = pstr.tile([8, 8], F32, tag="tr")
                nc.tensor.matmul(
                    out=gpsum,
                    lhsT=gsel_sb,
                    rhs=me2.rearrange("p a b -> p (a b)"),
                    start=True,
                    stop=True,
                )
                gp_v = gpsum.rearrange("p (a b) -> p a b", b=2)
                # group mean and E[x^2] (each channel stat counts 16 channels)
                gmv = spool.tile([8, 4, 2], F32, tag="gmv")
                nc.scalar.mul(out=gmv, in_=gp_v, mul=1.0 / GS)
                gvar = spool.tile([8, 4], F32, tag="gvar")
                nc.vector.tensor_tensor(
                    out=gvar, in0=gmv[:, :, 0], in1=gmv[:, :, 0], op=TT.mult
                )
                nc.vector.tensor_tensor(
                    out=gvar, in0=gmv[:, :, 1], in1=gvar, op=TT.subtract
                )
                gsd = spool.tile([8, 4], F32, tag="gsd")
                nc.scalar.activation(
                    out=gsd,
                    in_=gvar,
                    func=mybir.ActivationFunctionType.Sqrt,
                    bias=eps_sb[0:8, :],
                )
                grs = spool.tile([8, 4], F32, tag="grs")
                nc.vector.reciprocal(out=grs, in_=gsd)
                gac = spool.tile([8, 4, 2], F32, tag="gac")
                nc.vector.tensor_copy(out=gac[:, :, 0], in_=grs)
                nc.vector.tensor_tensor(
                    out=gac[:, :, 1], in0=gmv[:, :, 0], in1=grs, op=TT.mult
                )
                nc.scalar.mul(out=gac[:, :, 1], in_=gac[:, :, 1], mul=-1.0)

                # expand per c-tile to per-channel A (scale) / Bc (bias), then
                # xn = x*A + Bc
                xn = []
                for i in range(4):
                    epsum = pstr.tile([128, 2], F32, tag="tr")
                    nc.tensor.matmul(
                        out=epsum,
                        lhsT=gselt_sb[0:8, :],
                        rhs=gac[:, i, :],
                        start=True,
                        stop=True,
                    )
                    ab = abpool.tile([128, 2], F32, tag="ab")
                    nc.vector.tensor_tensor(
                        out=ab[:, 0:1],
                        in0=epsum[:, 0:1],
                        in1=gnw_sb[:, i : i + 1],
                        op=TT.mult,
                    )
                    nc.vector.tensor_tensor(
                        out=ab[:, 1:2],
                        in0=epsum[:, 1:2],
                        in1=gnw_sb[:, i : i + 1],
                        op=TT.mult,
                    )
                    nc.vector.tensor_tensor(
                        out=ab[:, 1:2],
                        in0=ab[:, 1:2],
                        in1=gnb_sb[:, i : i + 1],
                        op=TT.add,
                    )
                    xnt = xnp.tile([128, HW], F32R, tag="xn")
                    nc.vector.tensor_scalar(
                        out=xnt,
                        in0=xb[i],
                        scalar1=ab[:, 0:1],
                        scalar2=ab[:, 1:2],
                        op0=TT.mult,
                        op1=TT.add,
                    )
                    xn.append(xnt)

                # ======== q projection ========
                q = []
                for mi in range(4):
                    qt = qpool.tile([128, HW], F32R, tag="q")
                    for nh in range(2):
                        qps = psmm.tile([128, 512], F32, tag="mm")
                        for ki in range(4):
                            nc.tensor.matmul(
                                out=qps,
                                lhsT=wqt_sb[ki][:, 128 * mi : 128 * (mi + 1)],
                                rhs=xn[ki][:, 512 * nh : 512 * (nh + 1)],
                                start=(ki == 0),
                                stop=(ki == 3),
                            )
                        nc.scalar.copy(
                            out=qt[:, 512 * nh : 512 * (nh + 1)], in_=qps
                        )
                    q.append(qt)

                # ======== Phase T: LayerNorm + kv ========
                tb = tpool.tile([L, D], F32, tag="t")
                nc.sync.dma_start(out=tb, in_=t_l[b, :, :])
                stt = spool.tile([L, 3, 6], F32, tag="stt")
                for j in range(3):
                    nc.vector.bn_stats(
                        out=stt[:, j, :], in_=tb[:, 256 * j : 256 * (j + 1)]
                    )
                mvt = spool.tile([L, 2], F32, tag="mvt")
                nc.vector.bn_aggr(out=mvt, in_=stt)
                sdt = spool.tile([L, 1], F32, tag="sdt")
                nc.scalar.activation(
                    out=sdt,
                    in_=mvt[:, 1:2],
                    func=mybir.ActivationFunctionType.Sqrt,
                    bias=eps_sb[0:L, :],
                )
                rst = spool.tile([L, 1], F32, tag="rst")
                nc.vector.reciprocal(out=rst, in_=sdt)
                tn = tpool.tile([L, D], F32, tag="tn")
                nc.vector.tensor_scalar(
                    out=tn,
                    in0=tb,
                    scalar1=mvt[:, 0:1],
                    scalar2=rst,
                    op0=TT.subtract,
                    op1=TT.mult,
                )
                nc.vector.tensor_tensor(
                    out=tn, in0=tn, in1=lnw_sb[0:L, :], op=TT.mult
                )
                nc.vector.tensor_tensor(
                    out=tn, in0=tn, in1=lnb_sb[0:L, :], op=TT.add
                )

                tT = ttpool.tile([128, 6, L], F32R, tag="tT")
                for di in range(6):
                    tps = pstr.tile([128, L], F32, tag="tr")
                    nc.tensor.transpose(
                        tps, tn[:, 128 * di : 128 * (di + 1)], ident_sb[0:L, 0:L]
                    )
                    nc.scalar.copy(out=tT[:, di, :], in_=tps)

                kv = kvpool.tile([L, 2 * C], F32R, tag="kv")
                for nh in range(2):
                    kvps = psmm.tile([128, 512], F32, tag="mm")
                    for di in range(6):
                        nc.tensor.matmul(
                            out=kvps[0:L, :],
                            lhsT=tT[:, di, :],
                            rhs=wkvt_sb[di][:, 512 * nh : 512 * (nh + 1)],
                            start=(di == 0),
                            stop=(di == 5),
                        )
                    nc.scalar.copy(
                        out=kv[:, 512 * nh : 512 * (nh + 1)], in_=kvps[0:L, :]
                    )

                # k per head, transposed to [hd, L]; heads packed in pairs
                kT = []
                for hp in range(4):
                    kt = ktpool.tile([128, L], F32R, tag="kT")
                    kT.append(kt)
                for h in range(NH):
                    ktps = pstr.tile([128, L], F32, tag="tr")
                    nc.tensor.transpose(
                        ktps[0:HD, :],
                        kv[:, 128 * h : 128 * h + HD].bitcast(F32),
                        ident_sb[0:L, 0:L],
                    )
                    nc.scalar.copy(
                        out=kT[h // 2][64 * (h % 2) : 64 * (h % 2) + 64, :],
                        in_=ktps[0:HD, :],
                    )

                # ======== attention per head ========
                hsb = []
                for hp in range(4):
                    ht = hpool.tile([128, HW], F32R, tag="h")
                    hsb.append(ht)
                for h in range(NH):
                    hp, hh = h // 2, h % 2
                    ex = expool.tile([L, HW], F32R, tag="ex")
                    for nh in range(2):
                        atps = psmm.tile([128, 512], F32, tag="mm")
                        nc.tensor.matmul(
                            out=atps[0:L, :],
                            lhsT=kT[hp][64 * hh : 64 * hh + 64, :],
                            rhs=q[hp][64 * hh : 64 * hh + 64,
                                      512 * nh : 512 * (nh + 1)],
                            start=True,
                            stop=True,
                        )
                        nc.scalar.activation(
                            out=ex[:, 512 * nh : 512 * (nh + 1)],
                            in_=atps[0:L, :],
                            func=mybir.ActivationFunctionType.Exp,
                            scale=0.125,
                        )
                    rc = rcpool.tile([64, HW], F32, tag="rc")
                    for nh in range(2):
                        sl = slice(512 * nh, 512 * (nh + 1))
                        hups = psmm.tile([128, 512], F32, tag="mm")
                        nc.tensor.matmul(
                            out=hups[0:HD, :],
                            lhsT=kv[:, 128 * h + HD : 128 * (h + 1)],
                            rhs=ex[:, sl],
                            start=True,
                            stop=True,
                        )
                        dbps = psmm.tile([128, 512], F32, tag="mm")
                        nc.tensor.matmul(
                            out=dbps[0:HD, :],
                            lhsT=ones_sb[0:L, :],
                            rhs=ex[:, sl],
                            start=True,
                            stop=True,
                        )
                        if USE_DIVIDE:
                            nc.vector.tensor_tensor(
                                out=hsb[hp][64 * hh : 64 * hh + 64, sl],
                                in0=hups[0:HD, :],
                                in1=dbps[0:HD, :],
                                op=TT.divide,
                            )
                        else:
                            nc.vector.reciprocal_approx_fast(
                                out=rc[:, sl], in_=dbps[0:HD, :]
                            )
                            nc.vector.tensor_tensor(
                                out=hsb[hp][64 * hh : 64 * hh + 64, sl],
                                in0=hups[0:HD, :],
                                in1=rc[:, sl],
                                op=TT.mult,
                            )

                # ======== output projection + bias + residual ========
                for mi in range(4):
                    ob = opool.tile([128, HW], F32, tag="o")
                    for nh in range(2):
                        ops = psmm.tile([128, 512], F32, tag="mm")
                        for ki in range(4):
                            nc.tensor.matmul(
                                out=ops,
                                lhsT=wpt_sb[ki][:, 128 * mi : 128 * (mi + 1)],
                                rhs=hsb[ki][:, 512 * nh : 512 * (nh + 1)],
                                start=(ki == 0),
                                stop=(ki == 3),
                            )
                        sl = slice(512 * nh, 512 * (nh + 1))
                        nc.vector.tensor_scalar(
                            out=ob[:, sl],
                            in0=ops,
                            scalar1=bp_sb[:, mi : mi + 1],
                            scalar2=None,
                            op0=TT.add,
                        )
                        nc.gpsimd.tensor_tensor(
                            out=ob[:, sl], in0=ob[:, sl], in1=xb[mi][:, sl],
                            op=TT.add,
                        )
                    nc.sync.dma_start(
                        out=out_l[b, 128 * mi : 128 * (mi + 1), :], in_=ob
                    )

    nc.compile()
    return nc


def _host_constants(inputs):
    f = np.float32
    wqt = np.ascontiguousarray(np.asarray(inputs["Wq"], f).T)
    wkvt = np.ascontiguousarray(np.asarray(inputs["Wkv"], f).T)
    wpt = np.ascontiguousarray(np.asarray(inputs["Wp"], f).T)
    gnw4 = np.ascontiguousarray(np.asarray(inputs["gn_w"], f).reshape(4, 128).T)
    gnb4 = np.ascontiguousarray(np.asarray(inputs["gn_b"], f).reshape(4, 128).T)
    bp4 = np.ascontiguousarray(np.asarray(inputs["bp"], f).reshape(4, 128).T)
    lnw1 = np.asarray(inputs["ln_w"], f).reshape(1, D)
    lnb1 = np.asarray(inputs["ln_b"], f).reshape(1, D)
    gsel = np.kron(np.eye(8, dtype=f), np.ones((16, 1), f))
    gselt = np.ascontiguousarray(gsel.T)
    ident = np.eye(128, dtype=f)
    ones64 = np.ones((128, 64), f)
    return dict(
        wqt=wqt, wkvt=wkvt, wpt=wpt, gnw4=gnw4, gnb4=gnb4, bp4=bp4,
        lnw1=lnw1, lnb1=lnb1, gsel=gsel, gselt=gselt, ident=ident,
        ones64=ones64,
    )


def kernel(**inputs):
    global LAST_RESULTS
    if "nc" not in _CACHE:
        _CACHE["nc"] = _build_program()
    nc = _CACHE["nc"]

    consts = _host_constants(inputs)
    x = np.asarray(inputs["x"], np.float32).reshape(B, C, HW)
    t = np.asarray(inputs["t"], np.float32)

    in_maps = []
    for c in range(N_CORES):
        m = dict(consts)
        m["x_l"] = np.ascontiguousarray(x[BL * c : BL * (c + 1)])
        m["t_l"] = np.ascontiguousarray(t[BL * c : BL * (c + 1)])
        in_maps.append(m)

    res = run_bass_kernel_spmd(nc, in_maps, list(range(N_CORES)))
    LAST_RESULTS = res
    out = np.concatenate([res.results[c]["out_l"] for c in range(N_CORES)], axis=0)
    return out.reshape(B, C, H, W)


# revision 12
# speedup vs baseline: 1.1043x; 1.1043x over previous
"""CrossAttentionBlock TRN2 kernel.

Full inputs -> shard batch dim over 8 NeuronCores (data parallel, 4 batches
each) -> Bass/Tile kernel per core -> gather outputs.

Shapes (hardcoded): x [32,512,32,32] f32, t [32,77,768] f32,
Wq [512,512], Wkv [1024,768], Wp [512,512]; out [32,512,32,32].

Per-core plan (B_local=4, C=512, HW=1024, L=77, D=768, heads=8, hd=64):
  GroupNorm(32 groups): channel-on-partition layout [128,1024] x4 tiles;
    bn_stats per channel, group aggregation + per-channel expansion via tiny
    PE matmuls with constant selection matrices (host-provided).
  q = WqT.T @ xn        (f32r matmuls, N=512)
  LayerNorm(t): [77,768] row layout, then PE-transpose to [768,77].
  kv = t_lnT.T @ WkvT   -> [77, 1024] (l on partitions)
  per head h: k = kv[:, 128h:128h+64] transposed to [64,77] (PE),
    v stays [77(s), 64(c)] = natural lhsT for the AV matmul.
    attnT[s,t] = k.T @ q_head     ([77,512] x2, f32r)
    exp = Exp(0.125 * attnT)      (ACT, psum->sbuf; max-sub skipped, fp32 safe)
    hU[c,t]   = v.T @ exp         (f32r)  -- unnormalized
    den[c,t]  = ones.T @ exp      (f32r)  -- softmax denom broadcast to 64 rows
    h = hU * recip(den)           (DVE)
  out = WpT.T @ h + bp + x        (f32r matmuls; bias+residual on DVE/GPSIMD)
"""

import os
import sys

import numpy as np

for _p in ("/opt/trn_rl_repo", "/root/.axon_site/_ro/trn_rl_repo"):
    if _p not in sys.path and os.path.isdir(_p):
        sys.path.append(_p)

import concourse.bass as bass
import concourse.tile as tile
from concourse import bacc, mybir
from concourse.bass_utils import run_bass_kernel_spmd

F32 = mybir.dt.float32
F32R = mybir.dt.float32r
EPS = 1e-5

N_CORES = 8
B, C, H, W = 32, 512, 32, 32
HW = H * W
L, D = 77, 768
NH, HD = 8, 64
NG, GS = 32, 16  # groups, channels per group
BL = B // N_CORES  # local batches per core

LAST_RESULTS = None
_CACHE = {}
USE_DIVIDE = False


def R(ap):
    return ap.bitcast(F32R)


def _build_program():
    nc = bacc.Bacc("TRN2", target_bir_lowering=False, debug=False)

    x_l = nc.declare_dram_parameter("x_l", [BL, C, HW], F32, isOutput=False)
    t_l = nc.declare_dram_parameter("t_l", [BL, L, D], F32, isOutput=False)
    wqt = nc.declare_dram_parameter("wqt", [C, C], F32R, isOutput=False)
    wkvt = nc.declare_dram_parameter("wkvt", [D, 2 * C], F32R, isOutput=False)
    wpt = nc.declare_dram_parameter("wpt", [C, C], F32R, isOutput=False)
    gnw4 = nc.declare_dram_parameter("gnw4", [128, 4], F32, isOutput=False)
    gnb4 = nc.declare_dram_parameter("gnb4", [128, 4], F32, isOutput=False)
    bp4 = nc.declare_dram_parameter("bp4", [128, 4], F32, isOutput=False)
    lnw1 = nc.declare_dram_parameter("lnw1", [1, D], F32, isOutput=False)
    lnb1 = nc.declare_dram_parameter("lnb1", [1, D], F32, isOutput=False)
    gsel = nc.declare_dram_parameter("gsel", [128, 8], F32, isOutput=False)
    gselt = nc.declare_dram_parameter("gselt", [8, 128], F32, isOutput=False)
    ident = nc.declare_dram_parameter("ident", [128, 128], F32, isOutput=False)
    ones64 = nc.declare_dram_parameter("ones64", [128, 64], F32R, isOutput=False)
    out_l = nc.declare_dram_parameter("out_l", [BL, C, HW], F32, isOutput=True)

    TT = mybir.AluOpType

    with tile.TileContext(nc) as tc:
        with (
            tc.tile_pool(name="consts", bufs=1) as consts,
            tc.tile_pool(name="xp", bufs=9) as xp,
            tc.tile_pool(name="xnp", bufs=4) as xnp,
            tc.tile_pool(name="qp", bufs=5) as qpool,
            tc.tile_pool(name="hp", bufs=4) as hpool,
            tc.tile_pool(name="op", bufs=2) as opool,
            tc.tile_pool(name="tp", bufs=2) as tpool,
            tc.tile_pool(name="kvp", bufs=2) as kvpool,
            tc.tile_pool(name="ktp", bufs=6) as ktpool,
            tc.tile_pool(name="ttp", bufs=2) as ttpool,
            tc.tile_pool(name="exp", bufs=3) as expool,
            tc.tile_pool(name="rcp", bufs=2) as rcpool,
            tc.tile_pool(name="sp", bufs=4) as spool,
            tc.tile_pool(name="abp", bufs=6) as abpool,
            tc.tile_pool(name="psmm", bufs=6, space="PSUM") as psmm,
            tc.tile_pool(name="pstr", bufs=2, space="PSUM") as pstr,
        ):
            # ---- constants ----
            # ordering: small consts + wqt first (needed earliest), then
            # wkvt, wpt last, so batch-0 compute isn't gated on 5MB of DMA.
            gsel_sb = consts.tile([128, 8], F32, tag="gsel")
            nc.sync.dma_start(out=gsel_sb, in_=gsel[:, :])
            gselt_sb = consts.tile([8, 128], F32, tag="gselt")
            nc.sync.dma_start(out=gselt_sb, in_=gselt[:, :])
            ident_sb = consts.tile([128, 128], F32, tag="ident")
            nc.sync.dma_start(out=ident_sb, in_=ident[:, :])
            ones_sb = consts.tile([128, 64], F32R, tag="ones64")
            nc.sync.dma_start(out=ones_sb, in_=ones64[:, :])
            gnw_sb = consts.tile([128, 4], F32, tag="gnw")
            nc.sync.dma_start(out=gnw_sb, in_=gnw4[:, :])
            gnb_sb = consts.tile([128, 4], F32, tag="gnb")
            nc.sync.dma_start(out=gnb_sb, in_=gnb4[:, :])
            bp_sb = consts.tile([128, 4], F32, tag="bp")
            nc.sync.dma_start(out=bp_sb, in_=bp4[:, :])
            lnw_sb = consts.tile([128, D], F32, tag="lnw")
            nc.sync.dma_start(out=lnw_sb, in_=lnw1[:, :].to_broadcast([128, D]))
            lnb_sb = consts.tile([128, D], F32, tag="lnb")
            nc.sync.dma_start(out=lnb_sb, in_=lnb1[:, :].to_broadcast([128, D]))
            eps_sb = consts.tile([128, 1], F32, tag="eps")
            nc.vector.memset(eps_sb, EPS)
            wqt_sb = []
            for ki in range(4):
                tq = consts.tile([128, C], F32R, tag=f"wqt{ki}")
                nc.sync.dma_start(out=tq, in_=wqt[128 * ki : 128 * (ki + 1), :])
                wqt_sb.append(tq)
            wkvt_sb = []
            for di in range(6):
                tk = consts.tile([128, 2 * C], F32R, tag=f"wkvt{di}")
                nc.sync.dma_start(out=tk, in_=wkvt[128 * di : 128 * (di + 1), :])
                wkvt_sb.append(tk)
            wpt_sb = []
            for ki in range(4):
                tp_ = consts.tile([128, C], F32R, tag=f"wpt{ki}")
                nc.sync.dma_start(out=tp_, in_=wpt[128 * ki : 128 * (ki + 1), :])
                wpt_sb.append(tp_)

            for b in range(BL):
                # ======== Phase X: load x, GroupNorm stats ========
                xb = []
                for i in range(4):
                    xt = xp.tile([128, HW], F32, tag="x")
                    nc.sync.dma_start(
                        out=xt, in_=x_l[b, 128 * i : 128 * (i + 1), :]
                    )
                    xb.append(xt)

                mv = spool.tile([128, 4, 2], F32, tag="mv")
                for i in range(4):
                    st = spool.tile([128, 2, 6], F32, tag="bnst")
                    for j in range(2):
                        nc.vector.bn_stats(
                            out=st[:, j, :], in_=xb[i][:, 512 * j : 512 * (j + 1)]
                        )
                    nc.vector.bn_aggr(out=mv[:, i, :], in_=st)

                # me2[:, i, 0]=mean_c ; me2[:, i, 1]=var_c+mean_c^2 (=E[x^2]_c)
                me2 = spool.tile([128, 4, 2], F32, tag="me2")
                nc.vector.tensor_copy(out=me2[:, :, 0], in_=mv[:, :, 0])
                nc.vector.tensor_tensor(
                    out=me2[:, :, 1], in0=mv[:, :, 0], in1=mv[:, :, 0], op=TT.mult
                )
                nc.vector.tensor_tensor(
                    out=me2[:, :, 1], in0=me2[:, :, 1], in1=mv[:, :, 1], op=TT.add
                )
                gpsum = pstr.tile([8, 8], F32, tag="tr")
                nc.tensor.matmul(
                    out=gpsum,
                    lhsT=gsel_sb,
                    rhs=me2.rearrange("p a b -> p (a b)"),
                    start=True,
                    stop=True,
                )
                gp_v = gpsum.rearrange("p (a b) -> p a b", b=2)
                # group mean and E[x^2] (each channel stat counts 16 channels)
                gmv = spool.tile([8, 4, 2], F32, tag="gmv")
                nc.scalar.mul(out=gmv, in_=gp_v, mul=1.0 / GS)
                gvar = spool.tile([8, 4], F32, tag="gvar")
                nc.vector.tensor_tensor(
                    out=gvar, in0=gmv[:, :, 0], in1=gmv[:, :, 0], op=TT.mult
                )
                nc.vector.tensor_tensor(
                    out=gvar, in0=gmv[:, :, 1], in1=gvar, op=TT.subtract
                )
                gsd = spool.tile([8, 4], F32, tag="gsd")
                nc.scalar.activation(
                    out=gsd,
                    in_=gvar,
                    func=mybir.ActivationFunctionType.Sqrt,
                    bias=eps_sb[0:8, :],
                )
                grs = spool.tile([8, 4], F32, tag="grs")
                nc.vector.reciprocal(out=grs, in_=gsd)
                gac = spool.tile([8, 4, 2], F32, tag="gac")
                nc.vector.tensor_copy(out=gac[:, :, 0], in_=grs)
                nc.vector.tensor_tensor(
                    out=gac[:, :, 1], in0=gmv[:, :, 0], in1=grs, op=TT.mult
                )
                nc.scalar.mul(out=gac[:, :, 1], in_=gac[:, :, 1], mul=-1.0)

                # expand per c-tile to per-channel A (scale) / Bc (bias), then
                # xn = x*A + Bc
                xn = []
                for i in range(4):
                    epsum = pstr.tile([128, 2], F32, tag="tr")
                    nc.tensor.matmul(
                        out=epsum,
                        lhsT=gselt_sb[0:8, :],
                        rhs=gac[:, i, :],
                        start=True,
                        stop=True,
                    )
                    ab = abpool.tile([128, 2], F32, tag="ab")
                    nc.vector.tensor_tensor(
                        out=ab[:, 0:1],
                        in0=epsum[:, 0:1],
                        in1=gnw_sb[:, i : i + 1],
                        op=TT.mult,
                    )
                    nc.vector.tensor_tensor(
                        out=ab[:, 1:2],
                        in0=epsum[:, 1:2],
                        in1=gnw_sb[:, i : i + 1],
                        op=TT.mult,
                    )
                    nc.vector.tensor_tensor(
                        out=ab[:, 1:2],
                        in0=ab[:, 1:2],
                        in1=gnb_sb[:, i : i + 1],
                        op=TT.add,
                    )
                    xnt = xnp.tile([128, HW], F32R, tag="xn")
                    nc.vector.tensor_scalar(
                        out=xnt,
                        in0=xb[i],
                        scalar1=ab[:, 0:1],
                        scalar2=ab[:, 1:2],
                        op0=TT.mult,
                        op1=TT.add,
                    )
                    xn.append(xnt)

                # ======== q projection ========
                q = []
                for mi in range(4):
                    qt = qpool.tile([128, HW], F32R, tag="q")
                    for nh in range(2):
                        qps = psmm.tile([128, 512], F32, tag="mm")
                        for ki in range(4):
                            nc.tensor.matmul(
                                out=qps,
                                lhsT=wqt_sb[ki][:, 128 * mi : 128 * (mi + 1)],
                                rhs=xn[ki][:, 512 * nh : 512 * (nh + 1)],
                                start=(ki == 0),
                                stop=(ki == 3),
                            )
                        nc.scalar.copy(
                            out=qt[:, 512 * nh : 512 * (nh + 1)], in_=qps
                        )
                    q.append(qt)

                # ======== Phase T: LayerNorm + kv ========
                tb = tpool.tile([L, D], F32, tag="t")
                nc.sync.dma_start(out=tb, in_=t_l[b, :, :])
                stt = spool.tile([L, 3, 6], F32, tag="stt")
                for j in range(3):
                    nc.vector.bn_stats(
                        out=stt[:, j, :], in_=tb[:, 256 * j : 256 * (j + 1)]
                    )
                mvt = spool.tile([L, 2], F32, tag="mvt")
                nc.vector.bn_aggr(out=mvt, in_=stt)
                sdt = spool.tile([L, 1], F32, tag="sdt")
                nc.scalar.activation(
                    out=sdt,
                    in_=mvt[:, 1:2],
                    func=mybir.ActivationFunctionType.Sqrt,
                    bias=eps_sb[0:L, :],
                )
                rst = spool.tile([L, 1], F32, tag="rst")
                nc.vector.reciprocal(out=rst, in_=sdt)
                tn = tpool.tile([L, D], F32, tag="tn")
                nc.vector.tensor_scalar(
                    out=tn,
                    in0=tb,
                    scalar1=mvt[:, 0:1],
                    scalar2=rst,
                    op0=TT.subtract,
                    op1=TT.mult,
                )
                nc.vector.tensor_tensor(
                    out=tn, in0=tn, in1=lnw_sb[0:L, :], op=TT.mult
                )
                nc.vector.tensor_tensor(
                    out=tn, in0=tn, in1=lnb_sb[0:L, :], op=TT.add
                )

                tT = ttpool.tile([128, 6, L], F32R, tag="tT")
                for di in range(6):
                    tps = pstr.tile([128, L], F32, tag="tr")
                    nc.tensor.transpose(
                        tps, tn[:, 128 * di : 128 * (di + 1)], ident_sb[0:L, 0:L]
                    )
                    nc.scalar.copy(out=tT[:, di, :], in_=tps)

                kv = kvpool.tile([L, 2 * C], F32R, tag="kv")
                for nh in range(2):
                    kvps = psmm.tile([128, 512], F32, tag="mm")
                    for di in range(6):
                        nc.tensor.matmul(
                            out=kvps[0:L, :],
                            lhsT=tT[:, di, :],
                            rhs=wkvt_sb[di][:, 512 * nh : 512 * (nh + 1)],
                            start=(di == 0),
                            stop=(di == 5),
                        )
                    nc.scalar.copy(
                        out=kv[:, 512 * nh : 512 * (nh + 1)], in_=kvps[0:L, :]
                    )

                # k per head, transposed to [hd, L]; heads packed in pairs
                kT = []
                for hp in range(4):
                    kt = ktpool.tile([128, L], F32R, tag="kT")
                    kT.append(kt)
                for h in range(NH):
                    ktps = pstr.tile([128, L], F32, tag="tr")
                    nc.tensor.transpose(
                        ktps[0:HD, :],
                        kv[:, 128 * h : 128 * h + HD].bitcast(F32),
                        ident_sb[0:L, 0:L],
                    )
                    nc.scalar.copy(
                        out=kT[h // 2][64 * (h % 2) : 64 * (h % 2) + 64, :],
                        in_=ktps[0:HD, :],
                    )

                # ======== attention per head ========
                hsb = []
                for hp in range(4):
                    ht = hpool.tile([128, HW], F32R, tag="h")
                    hsb.append(ht)
                for h in range(NH):
                    hp, hh = h // 2, h % 2
                    ex = expool.tile([L, HW], F32R, tag="ex")
                    for nh in range(2):
                        atps = psmm.tile([128, 512], F32, tag="mm")
                        nc.tensor.matmul(
                            out=atps[0:L, :],
                            lhsT=kT[hp][64 * hh : 64 * hh + 64, :],
                            rhs=q[hp][64 * hh : 64 * hh + 64,
                                      512 * nh : 512 * (nh + 1)],
                            start=True,
                            stop=True,
                        )
                        nc.scalar.activation(
                            out=ex[:, 512 * nh : 512 * (nh + 1)],
                            in_=atps[0:L, :],
                            func=mybir.ActivationFunctionType.Exp,
                            scale=0.125,
                        )
                    rc = rcpool.tile([64, HW], F32, tag="rc")
                    for nh in range(2):
                        sl = slice(512 * nh, 512 * (nh + 1))
                        hups = psmm.tile([128, 512], F32, tag="mm")
                        nc.tensor.matmul(
                            out=hups[0:HD, :],
                            lhsT=kv[:, 128 * h + HD : 128 * (h + 1)],
                            rhs=ex[:, sl],
                            start=True,
                            stop=True,
                        )
                        dbps = psmm.tile([128, 512], F32, tag="mm")
                        nc.tensor.matmul(
                            out=dbps[0:HD, :],
                            lhsT=ones_sb[0:L, :],
                            rhs=ex[:, sl],
                            start=True,
                            stop=True,
                        )
                        if USE_DIVIDE:
                            nc.vector.tensor_tensor(
                                out=hsb[hp][64 * hh : 64 * hh + 64, sl],
                                in0=hups[0:HD, :],
                                in1=dbps[0:HD, :],
                                op=TT.divide,
                            )
                        else:
                            nc.vector.reciprocal_approx_fast(
                                out=rc[:, sl], in_=dbps[0:HD, :]
                            )
                            nc.vector.tensor_tensor(
                                out=hsb[hp][64 * hh : 64 * hh + 64, sl],
                                in0=hups[0:HD, :],
                                in1=rc[:, sl],
                                op=TT.mult,
                            )

                # ======== output projection + bias + residual ========
                for mi in range(4):
                    ob = opool.tile([128, HW], F32, tag="o")
                    for nh in range(2):
                        ops = psmm.tile([128, 512], F32, tag="mm")
                        for ki in range(4):
                            nc.tensor.matmul(
                                out=ops,
                                lhsT=wpt_sb[ki][:, 128 * mi : 128 * (mi + 1)],
                                rhs=hsb[ki][:, 512 * nh : 512 * (nh + 1)],
                                start=(ki == 0),
                                stop=(ki == 3),
                            )
                        sl = slice(512 * nh, 512 * (nh + 1))
                        nc.vector.tensor_scalar(
                            out=ob[:, sl],
                            in0=ops,
                            scalar1=bp_sb[:, mi : mi + 1],
                            scalar2=None,
                            op0=TT.add,
                        )
                        nc.gpsimd.tensor_tensor(
                            out=ob[:, sl], in0=ob[:, sl], in1=xb[mi][:, sl],
                            op=TT.add,
                        )
                    nc.sync.dma_start(
                        out=out_l[b, 128 * mi : 128 * (mi + 1), :], in_=ob
                    )

    nc.compile()
    return nc


def _host_constants(inputs):
    f = np.float32
    wqt = np.ascontiguousarray(np.asarray(inputs["Wq"], f).T)
    wkvt = np.ascontiguousarray(np.asarray(inputs["Wkv"], f).T)
    wpt = np.ascontiguousarray(np.asarray(inputs["Wp"], f).T)
    gnw4 = np.ascontiguousarray(np.asarray(inputs["gn_w"], f).reshape(4, 128).T)
    gnb4 = np.ascontiguousarray(np.asarray(inputs["gn_b"], f).reshape(4, 128).T)
    bp4 = np.ascontiguousarray(np.asarray(inputs["bp"], f).reshape(4, 128).T)
    lnw1 = np.asarray(inputs["ln_w"], f).reshape(1, D)
    lnb1 = np.asarray(inputs["ln_b"], f).reshape(1, D)
    gsel = np.kron(np.eye(8, dtype=f), np.ones((16, 1), f))
    gselt = np.ascontiguousarray(gsel.T)
    ident = np.eye(128, dtype=f)
    ones64 = np.ones((128, 64), f)
    return dict(
        wqt=wqt, wkvt=wkvt, wpt=wpt, gnw4=gnw4, gnb4=gnb4, bp4=bp4,
        lnw1=lnw1, lnb1=lnb1, gsel=gsel, gselt=gselt, ident=ident,
        ones64=ones64,
    )


def kernel(**inputs):
    global LAST_RESULTS
    if "nc" not in _CACHE:
        _CACHE["nc"] = _build_program()
    nc = _CACHE["nc"]

    consts = _host_constants(inputs)
    x = np.asarray(inputs["x"], np.float32).reshape(B, C, HW)
    t = np.asarray(inputs["t"], np.float32)

    in_maps = []
    for c in range(N_CORES):
        m = dict(consts)
        m["x_l"] = np.ascontiguousarray(x[BL * c : BL * (c + 1)])
        m["t_l"] = np.ascontiguousarray(t[BL * c : BL * (c + 1)])
        in_maps.append(m)

    res = run_bass_kernel_spmd(nc, in_maps, list(range(N_CORES)))
    LAST_RESULTS = res
    out = np.concatenate([res.results[c]["out_l"] for c in range(N_CORES)], axis=0)
    return out.reshape(B, C, H, W)
